# revision 8
# baseline (speedup 1.0000x reference)
"""Trainium2 Bass kernel for nn_Activation2d (anti-aliased activation):
   y = downsample2d(leaky_relu(upsample2d(x)))  on x [8, 64, 256, 256] fp32.

Algorithm: both resamplers are separable 1D kaiser-sinc filters, expressed as
banded matrices baked with edge-replication clamping:
  A [512,256] = up matrix (includes ratio factor 2), B [256,512] = down.
  y = B_h . lrelu(A_h X A_w^T) . B_w^T

All four matmul passes use the "windowed-rhs" form (the banded filter matrix
is the rhs with its nonzero column window sliced), which both transposes the
data each pass (so the next contraction lands on the partition axis) and
streams the minimum number of PE columns:
  P1 (contract h):  lhsT = X slice   [h, w-blk]   rhs = A^T[h-blk, n-win]
                    -> U  [w, n]      2x2 matmuls, N=262, f32r (x bitcast)
  P2 (contract w):  lhsT = U slice   [w, n-blk]   rhs = A^T[w-blk, w'-win]
                    -> V  [n, w']     2x4 matmuls, N=~261, fp16
  lrelu fused into the PSUM->SBUF copy (ACT Prelu / DVE-Pool scalar_tensor_tensor)
  P3 (contract n):  lhsT = L slice   [n, w'-blk]  rhs = B^T[n-blk, mh-win]
                    -> D' [w', h'']   4x4 matmuls, N=~70, fp16
  P4 (contract w'): lhsT = D' slice  [w', h''-blk] rhs = B^T[w'-blk, mw-win]
                    -> y  [h'', w'']  4x2 matmuls, N=~70, fp16

PE cost/image ~4824 cycles (vs 6776 for the form-D alternation), ~2.0us.
Engine copies balanced across ACT/DVE/Pool so none exceeds the PE time.
Input is consumed directly as float32r via AP.bitcast (f32r is fp32 bits with
reduced-mantissa PE consumption) -- no cast pass.

Sharding: pure data parallel over batch -- core b computes x[b] [64,256,256].
"""
import math
from contextlib import ExitStack

import numpy as np

import concourse.bass as bass
import concourse.bacc as bacc
import concourse.tile as tile
import concourse.mybir as mybir
from concourse.bass_utils import run_bass_kernel_spmd

RATIO = 2
KSIZE = 12
SLOPE = 0.2
H = W = 256
NCORES = 8

F32R = mybir.dt.float32r
F16 = mybir.dt.float16
F32 = mybir.dt.float32


# ----------------------------------------------------------------------------
# filter construction (mirrors the reference's kaiser_sinc_filter1d)
# ----------------------------------------------------------------------------
def _kaiser_sinc_filter1d(cutoff, half_width, kernel_size):
    half_size = kernel_size // 2
    delta_f = 4.0 * half_width
    A = 2.285 * (half_size - 1) * math.pi * delta_f + 7.95
    if A > 50.0:
        beta = 0.1102 * (A - 8.7)
    elif A >= 21.0:
        beta = 0.5842 * (A - 21.0) ** 0.4 + 0.07886 * (A - 21.0)
    else:
        beta = 0.0
    window = np.kaiser(kernel_size, beta)
    if kernel_size % 2 == 0:
        time = np.arange(-half_size, half_size) + 0.5
    else:
        time = np.arange(kernel_size) - half_size
    filt = 2.0 * cutoff * window * np.sinc(2.0 * cutoff * time)
    filt = filt / filt.sum()
    return filt.astype(np.float32)


def build_A(n_in=H):
    f = _kaiser_sinc_filter1d(0.5 / RATIO, 0.6 / RATIO, KSIZE).astype(np.float64)
    A = np.zeros((2 * n_in, n_in), np.float64)
    for t in range(n_in):
        for j in range(6):
            A[2 * t, np.clip(t + j - 3, 0, n_in - 1)] += 2.0 * f[2 * j]
            A[2 * t + 1, np.clip(t + j - 2, 0, n_in - 1)] += 2.0 * f[2 * j + 1]
    return A.astype(np.float32)


def build_B(n_out=H):
    f = _kaiser_sinc_filter1d(0.5 / RATIO, 0.6 / RATIO, KSIZE).astype(np.float64)
    B = np.zeros((n_out, 2 * n_out), np.float64)
    for m in range(n_out):
        for k in range(KSIZE):
            B[m, np.clip(2 * m + k - 5, 0, 2 * n_out - 1)] += f[k]
    return B.astype(np.float32)


def _nz_cols(mat, even=False):
    nz = np.nonzero(np.any(mat != 0.0, axis=0))[0]
    lo, hi = int(nz[0]), int(nz[-1]) + 1
    if even:
        lo -= lo % 2
        hi += hi % 2
    return lo, hi


# ----------------------------------------------------------------------------
# bass program
# ----------------------------------------------------------------------------
def build_nc(n_img=64, repeats=1, in_batch=4,
             eng_u="vector", eng_d="vector", eng_y="scalar",
             lrelu_eng=("scalar", "scalar"),
             skew=(0, 1, 2, 3),
             psum_bufs=(1, 2, 2),
             sbuf_bufs=(2, 3, 6, 3, 3)):
    A = build_A()          # [512, 256]
    B = build_B()          # [256, 512]
    AT = A.T.copy()        # [256, 512] rows h (or w), cols n (or w')
    BT = B.T.copy()        # [512, 256] rows n (or w'), cols m

    # windows: per 128-row block of AT / BT, nonzero column range
    w_up_r = [_nz_cols(AT[128 * b:128 * (b + 1)], even=True) for b in range(2)]
    w_up16 = [_nz_cols(AT[128 * b:128 * (b + 1)]) for b in range(2)]
    w_dn = [_nz_cols(BT[128 * k:128 * (k + 1)]) for k in range(4)]

    nc = bacc.Bacc("TRN2", target_bir_lowering=False, debug=False,
                   num_devices=NCORES)
    x_ap = nc.dram_tensor("x", [n_img, H, W], F32, kind="ExternalInput").ap()
    y_ap = nc.dram_tensor("y", [n_img, H, W], F32, kind="ExternalOutput").ap()

    at32_dram = nc.inline_tensor(np.ascontiguousarray(AT), name="at32")
    at16_dram = nc.inline_tensor(np.ascontiguousarray(AT).astype(np.float16),
                                 name="at16")
    bt16_dram = nc.inline_tensor(np.ascontiguousarray(BT).astype(np.float16),
                                 name="bt16")

    def eng(name):
        return {"vector": nc.vector, "scalar": nc.scalar, "pool": nc.gpsimd}[name]

    def lrelu_op(engine_name, dst, src):
        if engine_name == "scalar":
            nc.scalar.activation(dst, src,
                                 mybir.ActivationFunctionType.Prelu,
                                 alpha=SLOPE)
        else:
            # (v * SLOPE) max v  in one fused pass
            eng(engine_name).scalar_tensor_tensor(
                dst, src, SLOPE, src,
                mybir.AluOpType.mult, mybir.AluOpType.max)

    with tile.TileContext(nc) as tc, ExitStack() as ctx:
        cpool = ctx.enter_context(tc.tile_pool(name="consts", bufs=1))
        xpool = ctx.enter_context(tc.tile_pool(name="xin", bufs=sbuf_bufs[0]))
        upool = ctx.enter_context(tc.tile_pool(name="u", bufs=sbuf_bufs[1]))
        lpool = ctx.enter_context(tc.tile_pool(name="l", bufs=sbuf_bufs[2]))
        dpool = ctx.enter_context(tc.tile_pool(name="d", bufs=sbuf_bufs[3]))
        opool = ctx.enter_context(tc.tile_pool(name="o", bufs=sbuf_bufs[4]))
        # PSUM budget (8 banks): pp1 1x[128,1024] (2 banks) + pp2 2x[128,1024]
        # (4 banks) + pp34 2x[128,512] (2 banks, shared by P3-out and P4-out)
        pp1 = ctx.enter_context(tc.tile_pool(name="pp1", bufs=psum_bufs[0], space="PSUM"))
        pp2 = ctx.enter_context(tc.tile_pool(name="pp2", bufs=psum_bufs[1], space="PSUM"))
        pp34 = ctx.enter_context(tc.tile_pool(name="pp34", bufs=psum_bufs[2], space="PSUM"))

        # ---- constants -------------------------------------------------
        AT32 = []   # P1 rhs, f32r (DMA'd fp32 bits; PE rounds on consumption)
        AT16 = []   # P2 rhs
        for b in range(2):
            t32 = cpool.tile([128, 512], F32R, tag=f"at32_{b}")
            nc.sync.dma_start(
                t32[:], at32_dram.ap()[128 * b:128 * (b + 1), :].bitcast(F32R))
            AT32.append(t32)
            t16 = cpool.tile([128, 512], F16, tag=f"at16_{b}")
            nc.sync.dma_start(t16[:], at16_dram.ap()[128 * b:128 * (b + 1), :])
            AT16.append(t16)
        BT16 = []   # P3/P4 rhs
        for k in range(4):
            t16 = cpool.tile([128, 256], F16, tag=f"bt16_{k}")
            nc.sync.dma_start(t16[:], bt16_dram.ap()[128 * k:128 * (k + 1), :])
            BT16.append(t16)

        # ---- per-image pipeline ----------------------------------------
        xr_tiles = {}  # c -> (tile, col offset)
        state = {}     # c -> dict with u / L / d aps
        img_seq = [i for _ in range(repeats) for i in range(n_img)]
        n = len(img_seq)

        def stage1(idx):
            c = img_seq[idx]
            # -- input DMA: fp32, contiguous 1KB lines, batched ----------
            if idx % in_batch == 0:
                nb = min(in_batch, n - idx, n_img - c)
                xf = xpool.tile([128, nb * 512], F32R, tag="xf")
                src = x_ap[c:c + nb].rearrange(
                    "c (b p) w -> p c b w", p=128).bitcast(F32R)
                nc.sync.dma_start(
                    xf[:].rearrange("p (c b w) -> p c b w", c=nb, b=2), src)
                for i in range(nb):
                    xr_tiles[idx + i] = (xf, 512 * i)
            xf, off = xr_tiles.pop(idx)

            # -- P1: U[w-blk wb, n] = sum_h X[h, w] A^T[h, n] ------------
            # one [128,1024] psum tile (2 banks), one-instruction drain
            ps1 = pp1.tile([128, 1024], F32, tag="ps1")
            for wb in range(2):
                for i, hb in enumerate(range(2)):
                    lo, hi = w_up_r[hb]
                    nc.tensor.matmul(
                        ps1[:, 512 * wb + lo:512 * wb + hi],
                        xf[:, off + 256 * hb + 128 * wb:
                           off + 256 * hb + 128 * (wb + 1)],
                        AT32[hb][:, lo:hi],
                        start=(i == 0), stop=(i == 1),
                        skip_group_check=True,
                    )
            u = upool.tile([128, 1024], F16, tag="u")
            eng(eng_u).tensor_copy(u[:], ps1[:])
            state[idx] = {"u": u}

        def stage2(idx):
            # -- P2: V[n-blk ns, w'] = sum_w U[w, n] A^T[w, w'] ----------
            # two n-blocks packed per [128,1024] psum tile; lrelu fused in
            # the one-instruction drain
            u = state[idx]["u"]
            L = []
            for pair in range(2):
                ps = pp2.tile([128, 1024], F32, tag="ps2")
                for half in range(2):
                    ns = 2 * pair + half
                    for i, b in enumerate(range(2)):
                        lo, hi = w_up16[b]
                        nc.tensor.matmul(
                            ps[:, 512 * half + lo:512 * half + hi],
                            u[:, 512 * b + 128 * ns: 512 * b + 128 * (ns + 1)],
                            AT16[b][:, lo:hi],
                            start=(i == 0), stop=(i == 1),
                            skip_group_check=True,
                        )
                l = lpool.tile([128, 1024], F16, tag="l")
                lrelu_op(lrelu_eng[pair], l[:], ps[:])
                L.append(l)
            state[idx]["L"] = L

        def stage3(idx):
            # -- P3: D'[w'-blk j, mh] = sum_n L[n, w'] B^T[n, mh] --------
            L = state[idx].pop("L")
            tiles3 = [pp34.tile([128, 512], F32, tag="ps34", name="ps3a"),
                      pp34.tile([128, 512], F32, tag="ps34", name="ps3b")]
            seen = set()
            for k in range(4):          # k outer: start as soon as L[k//2] ready
                lo, hi = w_dn[k]
                lk = L[k // 2]
                koff = 512 * (k % 2)
                for j in range(4):
                    g = j // 2
                    col = 256 * (j % 2)
                    nc.tensor.matmul(
                        tiles3[g][:, col + lo:col + hi],
                        lk[:, koff + 128 * j:koff + 128 * (j + 1)],
                        BT16[k][:, lo:hi],
                        start=(g not in seen),
                        stop=(k == 3 and j >= 2 * g + 1),
                        skip_group_check=True,
                    )
                    seen.add(g)
            d = dpool.tile([128, 1024], F16, tag="d")
            for g in range(2):
                eng(eng_d).tensor_copy(d[:, 512 * g:512 * (g + 1)], tiles3[g][:])
            state[idx]["d"] = d

        def stage4(idx):
            # -- P4: y[mh-blk t, mw] = sum_w' D'[w', mh] B^T[w', mw] -----
            c = img_seq[idx]
            d = state[idx].pop("d")
            ps4 = pp34.tile([128, 512], F32, tag="ps34", name="ps4")
            first = True
            for j in range(4):          # j outer: start as soon as d[g] ready
                lo, hi = w_dn[j]
                for t in range(2):
                    nc.tensor.matmul(
                        ps4[:, 256 * t + lo:256 * t + hi],
                        d[:, 256 * j + 128 * t:256 * j + 128 * (t + 1)],
                        BT16[j][:, lo:hi],
                        start=first,
                        stop=(j == 3 and t == 1),
                        skip_group_check=True,
                    )
                    first = False
            o = opool.tile([128, 512], F32, tag="o")
            if eng_y == "scalar":
                nc.scalar.copy(o[:], ps4[:])
            else:
                eng(eng_y).tensor_copy(o[:], ps4[:])
            nc.sync.dma_start(
                y_ap[c].rearrange("(t p) w -> p t w", p=128),
                o[:].rearrange("p (t w) -> p t w", t=2))
            del state[idx]

        # software-pipelined emission
        s1, s2, s3, s4 = skew
        for s in range(n + max(skew)):
            if 0 <= s - s1 < n:
                stage1(s - s1)
            if 0 <= s - s3 < n:
                stage3(s - s3)
            if 0 <= s - s2 < n:
                stage2(s - s2)
            if 0 <= s - s4 < n:
                stage4(s - s4)

    nc.compile()
    return nc


_NC_CACHE = {}

# tuned configuration used by kernel()
BEST_CFG = dict()


def _get_nc(n_img, **overrides):
    cfg = dict(BEST_CFG, **overrides)
    key = (n_img, tuple(sorted((k, str(v)) for k, v in cfg.items())))
    if key not in _NC_CACHE:
        _NC_CACHE[key] = build_nc(n_img, **cfg)
    return _NC_CACHE[key]


def kernel(x: np.ndarray) -> np.ndarray:
    """x: [8, 64, 256, 256] fp32 -> y same shape."""
    x = np.asarray(x, dtype=np.float32)
    assert x.shape == (NCORES, 64, H, W), x.shape
    nc = _get_nc(64)
    in_maps = [{"x": x[b]} for b in range(NCORES)]
    res = run_bass_kernel_spmd(nc, in_maps, core_ids=list(range(NCORES)))
    return np.stack([res.results[b]["y"] for b in range(NCORES)], axis=0)
